# revision 7
# baseline (speedup 1.0000x reference)
"""Graphormer forward on 8 TRN2 NeuronCores (Bass/Tile).

Sharding: data-parallel over graphs, core c -> graphs 4c..4c+3.
Device works in transposed activation layout hT [D=256 (2 chunks of 128), SEQ=2048].

Host precomputes (exact math, no device collectives needed):
  - BN1 stats from X^T X (mean/var of x @ W_first^T are host-computable)
  - deg embedding rows hdeg = deg_emb[(adj!=0).sum(1)]
  - G = exp(attention bias) in [j, i] (transposed) per-core layout, bf16
  - BN2 (final batchnorm + leaky relu) applied on host to the device y2 output
  - v-projection bias folded into bo (softmax weights sum to 1)

Device per core:
  stage1: y = Wfirst' @ xT -> h = lrelu(y + b') + hdegT   [BN1 folded into W']
  2 transformer layers (attention with multiplicative exp-bias G, postnorm LNs)
  stage3: y2T = W_in @ hT + b_in -> DRAM out
"""

import numpy as np
import ml_dtypes

import concourse.bass as bass
import concourse.mybir as mybir
import concourse.tile as tile
from concourse import bacc
from concourse import bass_utils

F32 = mybir.dt.float32
BF16 = mybir.dt.bfloat16
AF = mybir.ActivationFunctionType
OP = mybir.AluOpType

B, N, DIN, D, H, L, DOUT = 32, 512, 256, 256, 8, 2, 256
DH = D // H          # 32
EPS = 1e-5
NCORES = 8
GPC = B // NCORES    # 4 graphs per core
SEQ = GPC * N        # 2048
SCALE = float(1.0 / np.sqrt(DH))
KC = D // 128        # 2 channel chunks
NCH = SEQ // 512     # 4 column chunks of 512

# bias column layout (DRAM "bcols" [128, NBC]):
#   bf: KC | qk: L*4 | bo: L*KC | b1: L*KC | b2: L*KC | bin: KC
def _bc_off(kind, l=0, mc=0):
    if kind == "bf":
        return mc
    if kind == "qk":
        return KC + l * 4 + mc
    if kind == "bo":
        return KC + L * 4 + l * KC + mc
    if kind == "b1":
        return KC + L * 4 + L * KC + l * KC + mc
    if kind == "b2":
        return KC + L * 4 + 2 * L * KC + l * KC + mc
    if kind == "bin":
        return KC + L * 4 + 3 * L * KC + mc
    raise KeyError(kind)


NBC = KC + L * 4 + 3 * L * KC + KC


def build():
    nc = bacc.Bacc("TRN2", target_bir_lowering=False, debug=False,
                   num_devices=NCORES)

    # ---- DRAM I/O ----
    xT_d = nc.dram_tensor("xT", [D, SEQ], F32, kind="ExternalInput")
    hdeg_d = nc.dram_tensor("hdegT", [D, SEQ], F32, kind="ExternalInput")
    g_d = nc.dram_tensor("gbias", [GPC, 2, 4, 128, 2048], BF16,
                         kind="ExternalInput")
    wf_d = nc.dram_tensor("wfirstT", [128, KC, D], F32, kind="ExternalInput")
    wqkv_d = nc.dram_tensor("wqkvT", [L, 128, KC, 3 * D], F32,
                            kind="ExternalInput")
    wo_d = nc.dram_tensor("woT", [L, 128, KC, D], F32, kind="ExternalInput")
    w1_d = nc.dram_tensor("w1T", [L, 128, KC, D], F32, kind="ExternalInput")
    w2_d = nc.dram_tensor("w2T", [L, 128, KC, D], F32, kind="ExternalInput")
    win_d = nc.dram_tensor("winT", [128, KC, D], F32, kind="ExternalInput")
    bc_d = nc.dram_tensor("bcols", [128, NBC], F32, kind="ExternalInput")
    lnp_d = nc.dram_tensor("lnp", [L, 128, KC, 4], F32, kind="ExternalInput")
    out_d = nc.dram_tensor("y2T", [D, SEQ], F32, kind="ExternalOutput")

    with tile.TileContext(nc) as tc:
        with tc.tile_pool(name="const", bufs=1) as constp, \
             tc.tile_pool(name="pers", bufs=1) as pers, \
             tc.tile_pool(name="big", bufs=4) as bigp, \
             tc.tile_pool(name="wpool", bufs=5) as wpool, \
             tc.tile_pool(name="gpool", bufs=5) as gpool, \
             tc.tile_pool(name="zpool", bufs=3) as zpool, \
             tc.tile_pool(name="small", bufs=4) as smallp, \
             tc.tile_pool(name="psS", bufs=2, space="PSUM") as psS, \
             tc.tile_pool(name="psB", bufs=2, space="PSUM") as psB, \
             tc.tile_pool(name="psOZ", bufs=1, space="PSUM") as psOZ:

            # ---- load constants / weights ----
            wf_sb = constp.tile([128, KC, D], F32, tag="wf")
            nc.sync.dma_start(wf_sb[:], wf_d.ap())
            wqkv_sb = [constp.tile([128, KC, 3 * D], F32, tag=f"wqkv{l}",
                                   name=f"wqkv{l}") for l in range(L)]
            wo_sb = [constp.tile([128, KC, D], F32, tag=f"wo{l}",
                                 name=f"wo{l}") for l in range(L)]
            w1_sb = [constp.tile([128, KC, D], F32, tag=f"w1{l}",
                                 name=f"w1{l}") for l in range(L)]
            w2_sb = [constp.tile([128, KC, D], F32, tag=f"w2{l}",
                                 name=f"w2{l}") for l in range(L)]
            for l in range(L):
                nc.sync.dma_start(wqkv_sb[l][:], wqkv_d.ap()[l])
                nc.sync.dma_start(wo_sb[l][:], wo_d.ap()[l])
                nc.sync.dma_start(w1_sb[l][:], w1_d.ap()[l])
                nc.sync.dma_start(w2_sb[l][:], w2_d.ap()[l])
            win_sb = constp.tile([128, KC, D], F32, tag="win")
            nc.sync.dma_start(win_sb[:], win_d.ap())
            bc_sb = constp.tile([128, NBC], F32, tag="bc")
            nc.sync.dma_start(bc_sb[:], bc_d.ap())
            lnp_sb = [constp.tile([128, KC, 4], F32, tag=f"lnp{l}",
                                  name=f"lnp{l}") for l in range(L)]
            for l in range(L):
                nc.sync.dma_start(lnp_sb[l][:], lnp_d.ap()[l])

            ones_col = constp.tile([128, 1], F32, tag="ones_col")
            nc.vector.memset(ones_col[:], 1.0)
            ones_mat_bf = constp.tile([128, 32], BF16, tag="ones_mat_bf")
            nc.vector.memset(ones_mat_bf[:], 1.0)
            ones_row128 = constp.tile([1, 128], F32, tag="ones_row128")
            nc.vector.memset(ones_row128[:], 1.0)
            dcol_row = constp.tile([1, 128], F32, tag="dcol_row")
            nc.vector.memset(dcol_row[:], float(D))
            eps2_row = constp.tile([1, 1], F32, tag="eps2_row")
            nc.vector.memset(eps2_row[:], float(D) * float(D) * EPS)

            def bcol(kind, l=0, mc=0):
                o = _bc_off(kind, l, mc)
                return bc_sb[:, o:o + 1]

            # ---- persistent activations ----
            h = [pers.tile([128, SEQ], F32, tag=f"h{k}", name=f"h{k}")
                 for k in range(KC)]
            qkT = [pers.tile([128, SEQ], BF16, tag=f"qk{m}", name=f"qk{m}")
                   for m in range(4)]
            v_norm = [pers.tile([128, D], BF16, tag=f"v{s}", name=f"v{s}")
                      for s in range(16)]
            # o_all doubles as the FFN hidden buffer (disjoint lifetimes)
            o_all = [pers.tile([128, SEQ], F32, tag=f"oall{k}",
                               name=f"oall{k}") for k in range(KC)]

            def proj_half(wT, mc, rhs_tiles, half):
                """psS tile [128, 1024] = wT[:, :, mc*128:..].T @ rhs[half]."""
                ps = psS.tile([128, 1024], F32, tag="psS", name="ps")
                msl = slice(mc * 128, (mc + 1) * 128)
                for nch in range(2):
                    base = half * 1024 + nch * 512
                    nsl = slice(base, base + 512)
                    psl = slice(nch * 512, (nch + 1) * 512)
                    for k in range(KC):
                        nc.tensor.matmul(ps[:, psl], lhsT=wT[:, k, msl],
                                         rhs=rhs_tiles[k][:, nsl],
                                         start=(k == 0), stop=(k == KC - 1))
                return ps

            def hsl(half):
                return slice(half * 1024, (half + 1) * 1024)

            # ================= stage 1 =================
            xin = [bigp.tile([128, SEQ], F32, tag="big", name="xin")
                   for _ in range(KC)]
            hdeg_sb = [bigp.tile([128, SEQ], F32, tag="big", name="hdeg")
                       for _ in range(KC)]
            for k in range(KC):
                nc.sync.dma_start(xin[k][:], xT_d.ap()[k * 128:(k + 1) * 128, :])
                nc.sync.dma_start(hdeg_sb[k][:],
                                  hdeg_d.ap()[k * 128:(k + 1) * 128, :])
            for mc in range(KC):
                for hf in range(2):
                    ps = proj_half(wf_sb[:], mc, xin, hf)
                    nc.scalar.activation(h[mc][:, hsl(hf)], ps[:], AF.Lrelu,
                                         bias=bcol("bf", mc=mc), alpha=0.01)
                    nc.vector.tensor_tensor(out=h[mc][:, hsl(hf)],
                                            in0=h[mc][:, hsl(hf)],
                                            in1=hdeg_sb[mc][:, hsl(hf)],
                                            op=OP.add)

            # ================= transformer layers =================
            for l in range(L):
                # ---- q^T, k^T (transposed layout, bf16) ----
                for mc in range(4):
                    for hf in range(2):
                        ps = proj_half(wqkv_sb[l][:], mc, h, hf)
                        nc.any.tensor_scalar(out=qkT[mc][:, hsl(hf)],
                                             in0=ps[:],
                                             scalar1=bcol("qk", l, mc),
                                             scalar2=None, op0=OP.add)
                # ---- v (seq-major, bf16; bias folded into bo on host) ----
                for st in range(16):
                    ssl = slice(st * 128, (st + 1) * 128)
                    ps = psB.tile([128, D], F32, tag="psB", name="psv")
                    for k in range(KC):
                        nc.tensor.matmul(ps[:], lhsT=h[k][:, ssl],
                                         rhs=wqkv_sb[l][:, k, 2 * D:3 * D],
                                         start=(k == 0), stop=(k == KC - 1))
                    nc.any.tensor_copy(out=v_norm[st][:], in_=ps[:])

                # ---- attention ----
                for g in range(GPC):
                    gs = slice(g * 512, (g + 1) * 512)
                    for hg in range(2):
                        oz = psOZ.tile([128, 1024], F32, tag="psOZ", name="oz")
                        for jt in range(4):
                            jsl = slice(g * 512 + jt * 128,
                                        g * 512 + jt * 128 + 128)
                            for rr in range(2):   # 2-head substeps
                                sc = psS.tile([128, 1024], F32, tag="psS",
                                              name="sc")
                                w_sb = wpool.tile([128, 1024], BF16, tag="w",
                                                  name="w_sb")
                                for q in range(2):
                                    r = rr * 2 + q
                                    qt = hg        # q^T tile index
                                    kt = 2 + hg    # k^T tile index
                                    rsl = slice(r * 32, (r + 1) * 32)
                                    nc.tensor.matmul(
                                        sc[:, q * 512:(q + 1) * 512],
                                        lhsT=qkT[kt][rsl, jsl],
                                        rhs=qkT[qt][rsl, gs],
                                        start=True, stop=True,
                                        tile_position=(r * 32, 0))
                                g_sb = gpool.tile([128, 1024], BF16, tag="g",
                                                  name="g_sb")
                                nc.sync.dma_start(
                                    g_sb[:],
                                    g_d.ap()[g, hg, jt][:,
                                        rr * 1024:(rr + 1) * 1024])
                                for q in range(2):
                                    qs = slice(q * 512, (q + 1) * 512)
                                    nc.scalar.activation(w_sb[:, qs],
                                                         sc[:, qs], AF.Exp,
                                                         scale=SCALE)
                                    nc.vector.tensor_tensor(out=w_sb[:, qs],
                                                            in0=w_sb[:, qs],
                                                            in1=g_sb[:, qs],
                                                            op=OP.mult)
                                for q in range(2):
                                    r = rr * 2 + q
                                    rssl = slice(q * 512, (q + 1) * 512)
                                    vsl = slice((hg * 4 + r) * 32,
                                                (hg * 4 + r) * 32 + 32)
                                    nc.tensor.matmul(
                                        oz[r * 32:(r + 1) * 32, 0:512],
                                        lhsT=v_norm[g * 4 + jt][:, vsl],
                                        rhs=w_sb[:, rssl],
                                        start=(jt == 0), stop=(jt == 3),
                                        tile_position=(0, r * 32))
                                    nc.tensor.matmul(
                                        oz[r * 32:(r + 1) * 32, 512:1024],
                                        lhsT=ones_mat_bf[:, 0:32],
                                        rhs=w_sb[:, rssl],
                                        start=(jt == 0), stop=(jt == 3),
                                        tile_position=(0, r * 32))
                        # normalize 4 heads: the Z matmul used a ones MATRIX
                        # lhsT, so oz[:, 512:1024] already holds Z replicated
                        # across each head's 32 rows.
                        zc = zpool.tile([128, 512], F32, tag="zc", name="zc")
                        nc.any.tensor_copy(out=zc[:], in_=oz[:, 512:1024])
                        rz = zpool.tile([128, 512], F32, tag="rz", name="rz")
                        nc.vector.reciprocal_approx_fast(out=rz[:], in_=zc[:])
                        nc.vector.tensor_tensor(out=o_all[hg][:, gs],
                                                in0=oz[:, 0:512], in1=rz[:],
                                                op=OP.mult)

                # ---- attn out proj + residual + LN1 ----
                for mc in range(KC):
                    for hf in range(2):
                        ps = proj_half(wo_sb[l][:], mc, o_all, hf)
                        nc.vector.scalar_tensor_tensor(
                            out=h[mc][:, hsl(hf)], in0=ps[:],
                            scalar=bcol("bo", l, mc),
                            in1=h[mc][:, hsl(hf)], op0=OP.add, op1=OP.add)
                layer_norm(nc, psOZ, psB, smallp, h, ones_col,
                           ones_row128, dcol_row, eps2_row, lnp_sb[l],
                           ln_idx=0)

                # ---- FFN (f1 reuses the o_all tiles) ----
                for mc in range(KC):
                    for hf in range(2):
                        ps = proj_half(w1_sb[l][:], mc, h, hf)
                        nc.vector.tensor_scalar(out=o_all[mc][:, hsl(hf)],
                                                in0=ps[:],
                                                scalar1=bcol("b1", l, mc),
                                                scalar2=0.0, op0=OP.add,
                                                op1=OP.max)
                for mc in range(KC):
                    for hf in range(2):
                        ps = proj_half(w2_sb[l][:], mc, o_all, hf)
                        nc.vector.scalar_tensor_tensor(
                            out=h[mc][:, hsl(hf)], in0=ps[:],
                            scalar=bcol("b2", l, mc),
                            in1=h[mc][:, hsl(hf)], op0=OP.add, op1=OP.add)
                layer_norm(nc, psOZ, psB, smallp, h, ones_col,
                           ones_row128, dcol_row, eps2_row, lnp_sb[l],
                           ln_idx=1)

            # ================= stage 3 =================
            for mc in range(KC):
                ot = bigp.tile([128, SEQ], F32, tag="big", name="ot")
                for hf in range(2):
                    ps = proj_half(win_sb[:], mc, h, hf)
                    nc.scalar.activation(ot[:, hsl(hf)], ps[:], AF.Identity,
                                         bias=bcol("bin", mc=mc))
                nc.sync.dma_start(out_d.ap()[mc * 128:(mc + 1) * 128, :], ot[:])

    nc.compile()
    return nc


def layer_norm(nc, psOZ, psB, smallp, h, ones_col, ones_row128, dcol_row,
               eps2_row, lnp_l, ln_idx):
    """Post-norm LN over the channel (partition) dim of hT [256, SEQ].

    Chunked by 512 tokens; fully on-chip. Per chunk:
      S, SS land on psum partition 0 (cols 0:512 / 512:1024)
      var' = D*SS - S^2 + D^2*eps ;  q = 1/sqrt(var')
      A = D*q  (K=1 matmul with lhsT = D)      [replicated 128 rows]
      B = S*q  (K=1 matmul with lhsT = 1)
      h = ((h*A) - B)*g + b
    """
    for nch in range(NCH):
        nsl = slice(nch * 512, (nch + 1) * 512)
        st = psOZ.tile([128, 1024], F32, tag="psOZ", name="st")
        xsq = smallp.tile([128, 512], F32, tag="lnxsq", name="xsq")
        for k in range(KC):
            nc.scalar.activation(xsq[:], h[k][:, nsl], AF.Square)
            nc.tensor.matmul(st[0:1, 0:512], lhsT=ones_col[:, 0:1],
                             rhs=h[k][:, nsl], start=(k == 0),
                             stop=(k == KC - 1))
            nc.tensor.matmul(st[0:1, 512:1024], lhsT=ones_col[:, 0:1],
                             rhs=xsq[:], start=(k == 0),
                             stop=(k == KC - 1))
        s2 = smallp.tile([1, 512], F32, tag="lns2", name="s2")
        nc.scalar.activation(s2[:], st[0:1, 0:512], AF.Square)
        varp = smallp.tile([1, 512], F32, tag="lnvarp", name="varp")
        nc.vector.scalar_tensor_tensor(out=varp[:], in0=st[0:1, 512:1024],
                                       scalar=float(D), in1=s2[:],
                                       op0=OP.mult, op1=OP.subtract)
        sd = smallp.tile([1, 512], F32, tag="lnsd", name="sd")
        nc.scalar.activation(sd[:], varp[:], AF.Sqrt, bias=eps2_row[0:1, 0:1])
        q = smallp.tile([1, 512], F32, tag="lnq", name="q")
        nc.vector.reciprocal_approx_fast(out=q[:], in_=sd[:])
        brow = smallp.tile([1, 512], F32, tag="lnbrow", name="brow")
        nc.vector.tensor_tensor(out=brow[:], in0=st[0:1, 0:512], in1=q[:],
                                op=OP.mult)
        a_ps = psB.tile([128, 512], F32, tag="psB", name="a_ps")
        nc.tensor.matmul(a_ps[:], lhsT=dcol_row[0:1, :], rhs=q[:],
                         start=True, stop=True)
        b_ps = psB.tile([128, 512], F32, tag="psB", name="b_ps")
        nc.tensor.matmul(b_ps[:], lhsT=ones_row128[0:1, :], rhs=brow[:],
                         start=True, stop=True)
        # apply: h = ((h * A) - B) * g + b
        for k in range(KC):
            t = smallp.tile([128, 512], F32, tag="lnt", name="t")
            nc.vector.tensor_tensor(out=t[:], in0=h[k][:, nsl], in1=a_ps[:],
                                    op=OP.mult)
            nc.vector.tensor_tensor(out=t[:], in0=t[:], in1=b_ps[:],
                                    op=OP.subtract)
            nc.scalar.activation(h[k][:, nsl], t[:], AF.Identity,
                                 bias=lnp_l[:, k, 2 * ln_idx + 1:
                                            2 * ln_idx + 2],
                                 scale=lnp_l[:, k, 2 * ln_idx:2 * ln_idx + 1])


# ================= host side =================

_COMPILED = None


def _get_compiled():
    global _COMPILED
    if _COMPILED is None:
        _COMPILED = build()
    return _COMPILED


def prepare_inputs(inputs):
    """Returns (in_maps, bn2_params) for the 8 cores."""
    f32 = np.float32
    x = np.asarray(inputs["x"], f32)
    adj = np.asarray(inputs["adj_fc"])
    spd = np.asarray(inputs["spd_dist"])
    W_first = np.asarray(inputs["W_first"], f32)
    b_first = np.asarray(inputs["b_first"], f32)
    bn1_g = np.asarray(inputs["bn1_g"], f32)
    bn1_b = np.asarray(inputs["bn1_b"], f32)
    deg_emb = np.asarray(inputs["deg_emb"], f32)
    spd_emb = np.asarray(inputs["spd_emb"], f32)
    Wqkv = np.asarray(inputs["Wqkv"], f32)
    bqkv = np.asarray(inputs["bqkv"], f32)
    Wo = np.asarray(inputs["Wo"], f32)
    bo = np.asarray(inputs["bo"], f32)
    ln1_g = np.asarray(inputs["ln1_g"], f32)
    ln1_b = np.asarray(inputs["ln1_b"], f32)
    W1 = np.asarray(inputs["W1"], f32)
    b1 = np.asarray(inputs["b1"], f32)
    W2 = np.asarray(inputs["W2"], f32)
    b2 = np.asarray(inputs["b2"], f32)
    ln2_g = np.asarray(inputs["ln2_g"], f32)
    ln2_b = np.asarray(inputs["ln2_b"], f32)
    W_in = np.asarray(inputs["W_in"], f32)
    b_in = np.asarray(inputs["b_in"], f32)

    # ---- BN1 stats (exact, host) ----
    Xd = x.astype(np.float64)
    M = Xd.shape[0]
    mu_x = Xd.mean(0)
    C = (Xd.T @ Xd) / M
    Wd = W_first.astype(np.float64)
    m1 = Wd @ mu_x + b_first
    e2 = np.einsum("oc,cd,od->o", Wd, C, Wd)
    v1 = e2 - (Wd @ mu_x) ** 2
    s1 = (bn1_g / np.sqrt(v1 + EPS)).astype(f32)
    Wf_eff = (W_first * s1[:, None]).astype(f32)
    bf_eff = (b_first * s1 + bn1_b - m1.astype(f32) * s1).astype(f32)

    # ---- deg embedding ----
    deg = (adj != 0).sum(1)
    hdeg = deg_emb[deg]                                # [B, N, D]

    # ---- fold v-bias through Wo into bo (softmax rows sum to 1) ----
    bv = bqkv[:, 2 * D:3 * D]                          # [L, D]
    bo_eff = bo + np.einsum("lod,ld->lo", Wo, bv)

    def pack_wT(W):
        WT = np.ascontiguousarray(W.T)                 # [din, dout]
        return WT.reshape(KC, 128, W.shape[0]).transpose(1, 0, 2).copy()

    wfirstT = pack_wT(Wf_eff)
    wqkvT = np.stack([pack_wT(Wqkv[l]) for l in range(L)])
    woT = np.stack([pack_wT(Wo[l]) for l in range(L)])
    w1T = np.stack([pack_wT(W1[l]) for l in range(L)])
    w2T = np.stack([pack_wT(W2[l]) for l in range(L)])
    winT = pack_wT(W_in)

    bcols = np.zeros((128, NBC), f32)
    for mc in range(KC):
        bcols[:, _bc_off("bf", mc=mc)] = bf_eff[mc * 128:(mc + 1) * 128]
        bcols[:, _bc_off("bin", mc=mc)] = b_in[mc * 128:(mc + 1) * 128]
    for l in range(L):
        for mc in range(4):
            bcols[:, _bc_off("qk", l, mc)] = bqkv[l][mc * 128:(mc + 1) * 128]
        for mc in range(KC):
            bcols[:, _bc_off("bo", l, mc)] = bo_eff[l][mc * 128:(mc + 1) * 128]
            bcols[:, _bc_off("b1", l, mc)] = b1[l][mc * 128:(mc + 1) * 128]
            bcols[:, _bc_off("b2", l, mc)] = b2[l][mc * 128:(mc + 1) * 128]

    lnp = np.zeros((L, 128, KC, 4), f32)
    for l in range(L):
        for k in range(KC):
            sl = slice(k * 128, (k + 1) * 128)
            lnp[l, :, k, 0] = ln1_g[l][sl]
            lnp[l, :, k, 1] = ln1_b[l][sl]
            lnp[l, :, k, 2] = ln2_g[l][sl]
            lnp[l, :, k, 3] = ln2_b[l][sl]

    shared = {
        "wfirstT": wfirstT, "wqkvT": wqkvT, "woT": woT, "w1T": w1T,
        "w2T": w2T, "winT": winT, "bcols": bcols, "lnp": lnp,
    }

    # ---- G = exp(bias), [j, i]-transposed, per-core layout ----
    neg = spd < 0
    idx = np.where(neg, 0, spd)
    expT = np.exp(spd_emb)                             # [100, H]
    einv = f32(np.exp(-1.0))

    in_maps = []
    for c in range(NCORES):
        gsl = slice(4 * c * N, 4 * (c + 1) * N)
        xT = np.ascontiguousarray(x[gsl].T)
        hdegT = np.ascontiguousarray(
            hdeg[4 * c:4 * (c + 1)].reshape(SEQ, D).T)
        gb = np.empty((GPC, 2, 4, 128, 2048), ml_dtypes.bfloat16)
        for gl in range(GPC):
            for hh in range(H):
                src = 8 * gl + hh
                val = expT[idx[src], c]                # [N_i, N_j]
                val = np.where(neg[src], einv, val)
                vT = val.T                             # [j, i]
                hg, r = hh // 4, hh % 4
                for jt in range(4):
                    gb[gl, hg, jt, :, r * 512:(r + 1) * 512] = \
                        vT[jt * 128:(jt + 1) * 128, :]
        m = {"xT": xT, "hdegT": hdegT, "gbias": gb}
        m.update(shared)
        in_maps.append(m)

    bn2 = (np.asarray(inputs["bn2_g"], f32), np.asarray(inputs["bn2_b"], f32))
    return in_maps, bn2


def finish_host(results, bn2):
    g2, b2 = bn2
    y2 = np.concatenate([np.ascontiguousarray(r["y2T"].T) for r in results])
    yd = y2.astype(np.float64)
    m = yd.mean(0)
    v = yd.var(0)
    s = (g2 / np.sqrt(v + EPS)).astype(np.float32)
    t = (b2 - m.astype(np.float32) * s)
    out = y2 * s + t
    return np.where(out >= 0, out, np.float32(0.01) * out).astype(np.float32)


def kernel(**inputs):
    nc = _get_compiled()
    in_maps, bn2 = prepare_inputs(inputs)
    res = bass_utils.run_bass_kernel_spmd(
        nc, in_maps, core_ids=list(range(NCORES)))
    return finish_host(res.results, bn2)


# revision 8
# speedup vs baseline: 1.0301x; 1.0301x over previous
"""Graphormer forward on 8 TRN2 NeuronCores (Bass/Tile).

Sharding: data-parallel over graphs, core c -> graphs 4c..4c+3.
Device works in transposed activation layout hT [D=256 (2 chunks of 128), SEQ=2048].

Host precomputes (exact math, no device collectives needed):
  - BN1 stats from X^T X (mean/var of x @ W_first^T are host-computable)
  - deg embedding rows hdeg = deg_emb[(adj!=0).sum(1)]
  - G = exp(attention bias) in [j, i] (transposed) per-core layout, bf16
  - BN2 (final batchnorm + leaky relu) applied on host to the device y2 output
  - v-projection bias folded into bo (softmax weights sum to 1)

Device per core:
  stage1: y = Wfirst' @ xT -> h = lrelu(y + b') + hdegT   [BN1 folded into W']
  2 transformer layers (attention with multiplicative exp-bias G, postnorm LNs)
  stage3: y2T = W_in @ hT + b_in -> DRAM out
"""

import numpy as np
import ml_dtypes

import concourse.bass as bass
import concourse.mybir as mybir
import concourse.tile as tile
from concourse import bacc
from concourse import bass_utils

F32 = mybir.dt.float32
BF16 = mybir.dt.bfloat16
AF = mybir.ActivationFunctionType
OP = mybir.AluOpType

B, N, DIN, D, H, L, DOUT = 32, 512, 256, 256, 8, 2, 256
DH = D // H          # 32
EPS = 1e-5
NCORES = 8
GPC = B // NCORES    # 4 graphs per core
SEQ = GPC * N        # 2048
SCALE = float(1.0 / np.sqrt(DH))
KC = D // 128        # 2 channel chunks
NCH = SEQ // 512     # 4 column chunks of 512

# bias column layout (DRAM "bcols" [128, NBC]):
#   bf: KC | qk: L*4 | bo: L*KC | b1: L*KC | b2: L*KC | bin: KC
def _bc_off(kind, l=0, mc=0):
    if kind == "bf":
        return mc
    if kind == "qk":
        return KC + l * 4 + mc
    if kind == "bo":
        return KC + L * 4 + l * KC + mc
    if kind == "b1":
        return KC + L * 4 + L * KC + l * KC + mc
    if kind == "b2":
        return KC + L * 4 + 2 * L * KC + l * KC + mc
    if kind == "bin":
        return KC + L * 4 + 3 * L * KC + mc
    raise KeyError(kind)


NBC = KC + L * 4 + 3 * L * KC + KC


def build():
    nc = bacc.Bacc("TRN2", target_bir_lowering=False, debug=False,
                   num_devices=NCORES)

    # ---- DRAM I/O ----
    xT_d = nc.dram_tensor("xT", [D, SEQ], F32, kind="ExternalInput")
    hdeg_d = nc.dram_tensor("hdegT", [D, SEQ], F32, kind="ExternalInput")
    g_d = nc.dram_tensor("gbias", [GPC, 2, 4, 128, 2048], BF16,
                         kind="ExternalInput")
    wf_d = nc.dram_tensor("wfirstT", [128, KC, D], F32, kind="ExternalInput")
    wqkv_d = nc.dram_tensor("wqkvT", [L, 128, KC, 3 * D], F32,
                            kind="ExternalInput")
    wo_d = nc.dram_tensor("woT", [L, 128, KC, D], F32, kind="ExternalInput")
    w1_d = nc.dram_tensor("w1T", [L, 128, KC, D], F32, kind="ExternalInput")
    w2_d = nc.dram_tensor("w2T", [L, 128, KC, D], F32, kind="ExternalInput")
    win_d = nc.dram_tensor("winT", [128, KC, D], F32, kind="ExternalInput")
    bc_d = nc.dram_tensor("bcols", [128, NBC], F32, kind="ExternalInput")
    lnp_d = nc.dram_tensor("lnp", [L, 128, KC, 4], F32, kind="ExternalInput")
    out_d = nc.dram_tensor("y2T", [D, SEQ], F32, kind="ExternalOutput")

    with tile.TileContext(nc) as tc:
        with tc.tile_pool(name="const", bufs=1) as constp, \
             tc.tile_pool(name="pers", bufs=1) as pers, \
             tc.tile_pool(name="big", bufs=4) as bigp, \
             tc.tile_pool(name="wpool", bufs=5) as wpool, \
             tc.tile_pool(name="gpool", bufs=5) as gpool, \
             tc.tile_pool(name="zpool", bufs=3) as zpool, \
             tc.tile_pool(name="small", bufs=4) as smallp, \
             tc.tile_pool(name="psS", bufs=2, space="PSUM") as psS, \
             tc.tile_pool(name="psB", bufs=2, space="PSUM") as psB, \
             tc.tile_pool(name="psOZ", bufs=1, space="PSUM") as psOZ:

            # ---- load constants / weights ----
            wf_sb = constp.tile([128, KC, D], F32, tag="wf")
            nc.sync.dma_start(wf_sb[:], wf_d.ap())
            wqkv_sb = [constp.tile([128, KC, 3 * D], F32, tag=f"wqkv{l}",
                                   name=f"wqkv{l}") for l in range(L)]
            wo_sb = [constp.tile([128, KC, D], F32, tag=f"wo{l}",
                                 name=f"wo{l}") for l in range(L)]
            w1_sb = [constp.tile([128, KC, D], F32, tag=f"w1{l}",
                                 name=f"w1{l}") for l in range(L)]
            w2_sb = [constp.tile([128, KC, D], F32, tag=f"w2{l}",
                                 name=f"w2{l}") for l in range(L)]
            for l in range(L):
                nc.sync.dma_start(wqkv_sb[l][:], wqkv_d.ap()[l])
                nc.sync.dma_start(wo_sb[l][:], wo_d.ap()[l])
                nc.sync.dma_start(w1_sb[l][:], w1_d.ap()[l])
                nc.sync.dma_start(w2_sb[l][:], w2_d.ap()[l])
            win_sb = constp.tile([128, KC, D], F32, tag="win")
            nc.sync.dma_start(win_sb[:], win_d.ap())
            bc_sb = constp.tile([128, NBC], F32, tag="bc")
            nc.sync.dma_start(bc_sb[:], bc_d.ap())
            lnp_sb = [constp.tile([128, KC, 4], F32, tag=f"lnp{l}",
                                  name=f"lnp{l}") for l in range(L)]
            for l in range(L):
                nc.sync.dma_start(lnp_sb[l][:], lnp_d.ap()[l])

            ones_col = constp.tile([128, 1], F32, tag="ones_col")
            nc.vector.memset(ones_col[:], 1.0)
            ones_mat_bf = constp.tile([128, 32], BF16, tag="ones_mat_bf")
            nc.vector.memset(ones_mat_bf[:], 1.0)
            ones_row128 = constp.tile([1, 128], F32, tag="ones_row128")
            nc.vector.memset(ones_row128[:], 1.0)
            dcol_row = constp.tile([1, 128], F32, tag="dcol_row")
            nc.vector.memset(dcol_row[:], float(D))
            eps2_row = constp.tile([1, 1], F32, tag="eps2_row")
            nc.vector.memset(eps2_row[:], float(D) * float(D) * EPS)

            def bcol(kind, l=0, mc=0):
                o = _bc_off(kind, l, mc)
                return bc_sb[:, o:o + 1]

            # ---- persistent activations ----
            h = [pers.tile([128, SEQ], F32, tag=f"h{k}", name=f"h{k}")
                 for k in range(KC)]
            qkT = [pers.tile([128, SEQ], BF16, tag=f"qk{m}", name=f"qk{m}")
                   for m in range(4)]
            v_norm = [pers.tile([128, D], BF16, tag=f"v{s}", name=f"v{s}")
                      for s in range(16)]
            # o_all doubles as the FFN hidden buffer (disjoint lifetimes)
            o_all = [pers.tile([128, SEQ], F32, tag=f"oall{k}",
                               name=f"oall{k}") for k in range(KC)]

            def proj_half(wT, mc, rhs_tiles, half):
                """psS tile [128, 1024] = wT[:, :, mc*128:..].T @ rhs[half]."""
                ps = psS.tile([128, 1024], F32, tag="psS", name="ps")
                msl = slice(mc * 128, (mc + 1) * 128)
                for nch in range(2):
                    base = half * 1024 + nch * 512
                    nsl = slice(base, base + 512)
                    psl = slice(nch * 512, (nch + 1) * 512)
                    for k in range(KC):
                        nc.tensor.matmul(ps[:, psl], lhsT=wT[:, k, msl],
                                         rhs=rhs_tiles[k][:, nsl],
                                         start=(k == 0), stop=(k == KC - 1))
                return ps

            def hsl(half):
                return slice(half * 1024, (half + 1) * 1024)

            # ================= stage 1 =================
            xin = [bigp.tile([128, SEQ], F32, tag="big", name="xin")
                   for _ in range(KC)]
            hdeg_sb = [bigp.tile([128, SEQ], F32, tag="big", name="hdeg")
                       for _ in range(KC)]
            for k in range(KC):
                nc.sync.dma_start(xin[k][:], xT_d.ap()[k * 128:(k + 1) * 128, :])
                nc.sync.dma_start(hdeg_sb[k][:],
                                  hdeg_d.ap()[k * 128:(k + 1) * 128, :])
            for mc in range(KC):
                for hf in range(2):
                    ps = proj_half(wf_sb[:], mc, xin, hf)
                    nc.scalar.activation(h[mc][:, hsl(hf)], ps[:], AF.Lrelu,
                                         bias=bcol("bf", mc=mc), alpha=0.01)
                    nc.vector.tensor_tensor(out=h[mc][:, hsl(hf)],
                                            in0=h[mc][:, hsl(hf)],
                                            in1=hdeg_sb[mc][:, hsl(hf)],
                                            op=OP.add)

            # ================= transformer layers =================
            for l in range(L):
                # ---- q^T, k^T (transposed layout, bf16) ----
                for mc in range(4):
                    for hf in range(2):
                        ps = proj_half(wqkv_sb[l][:], mc, h, hf)
                        nc.any.tensor_scalar(out=qkT[mc][:, hsl(hf)],
                                             in0=ps[:],
                                             scalar1=bcol("qk", l, mc),
                                             scalar2=None, op0=OP.add)
                # ---- v (seq-major, bf16; bias folded into bo on host) ----
                for st in range(16):
                    ssl = slice(st * 128, (st + 1) * 128)
                    ps = psB.tile([128, D], F32, tag="psB", name="psv")
                    for k in range(KC):
                        nc.tensor.matmul(ps[:], lhsT=h[k][:, ssl],
                                         rhs=wqkv_sb[l][:, k, 2 * D:3 * D],
                                         start=(k == 0), stop=(k == KC - 1))
                    nc.any.tensor_copy(out=v_norm[st][:], in_=ps[:])

                # ---- attention ----
                for g in range(GPC):
                    gs = slice(g * 512, (g + 1) * 512)
                    for hg in range(2):
                        oz = psOZ.tile([128, 1024], F32, tag="psOZ", name="oz")
                        for jt in range(4):
                            jsl = slice(g * 512 + jt * 128,
                                        g * 512 + jt * 128 + 128)
                            for rr in range(2):   # 2-head substeps
                                sc = psS.tile([128, 1024], F32, tag="psS",
                                              name="sc")
                                w_sb = wpool.tile([128, 1024], BF16, tag="w",
                                                  name="w_sb")
                                for q in range(2):
                                    r = rr * 2 + q
                                    qt = hg        # q^T tile index
                                    kt = 2 + hg    # k^T tile index
                                    rsl = slice(r * 32, (r + 1) * 32)
                                    nc.tensor.matmul(
                                        sc[:, q * 512:(q + 1) * 512],
                                        lhsT=qkT[kt][rsl, jsl],
                                        rhs=qkT[qt][rsl, gs],
                                        start=True, stop=True,
                                        tile_position=(r * 32, 0))
                                nc.scalar.activation(w_sb[:], sc[:], AF.Exp,
                                                     scale=SCALE)
                                g_sb = gpool.tile([128, 1024], BF16, tag="g",
                                                  name="g_sb")
                                nc.sync.dma_start(
                                    g_sb[:],
                                    g_d.ap()[g, hg, jt][:,
                                        rr * 1024:(rr + 1) * 1024])
                                nc.vector.tensor_tensor(out=w_sb[:],
                                                        in0=w_sb[:],
                                                        in1=g_sb[:],
                                                        op=OP.mult)
                                for q in range(2):
                                    r = rr * 2 + q
                                    rssl = slice(q * 512, (q + 1) * 512)
                                    vsl = slice((hg * 4 + r) * 32,
                                                (hg * 4 + r) * 32 + 32)
                                    nc.tensor.matmul(
                                        oz[r * 32:(r + 1) * 32, 0:512],
                                        lhsT=v_norm[g * 4 + jt][:, vsl],
                                        rhs=w_sb[:, rssl],
                                        start=(jt == 0), stop=(jt == 3),
                                        tile_position=(0, r * 32))
                                    nc.tensor.matmul(
                                        oz[r * 32:(r + 1) * 32, 512:1024],
                                        lhsT=ones_mat_bf[:, 0:32],
                                        rhs=w_sb[:, rssl],
                                        start=(jt == 0), stop=(jt == 3),
                                        tile_position=(0, r * 32))
                        # normalize 4 heads: the Z matmul used a ones MATRIX
                        # lhsT, so oz[:, 512:1024] already holds Z replicated
                        # across each head's 32 rows.
                        zc = zpool.tile([128, 512], F32, tag="zc", name="zc")
                        nc.any.tensor_copy(out=zc[:], in_=oz[:, 512:1024])
                        rz = zpool.tile([128, 512], F32, tag="rz", name="rz")
                        nc.vector.reciprocal_approx_fast(out=rz[:], in_=zc[:])
                        nc.vector.tensor_tensor(out=o_all[hg][:, gs],
                                                in0=oz[:, 0:512], in1=rz[:],
                                                op=OP.mult)

                # ---- attn out proj + residual + LN1 ----
                for mc in range(KC):
                    for hf in range(2):
                        ps = proj_half(wo_sb[l][:], mc, o_all, hf)
                        nc.vector.scalar_tensor_tensor(
                            out=h[mc][:, hsl(hf)], in0=ps[:],
                            scalar=bcol("bo", l, mc),
                            in1=h[mc][:, hsl(hf)], op0=OP.add, op1=OP.add)
                layer_norm(nc, psOZ, psB, smallp, h, ones_col,
                           ones_row128, dcol_row, eps2_row, lnp_sb[l],
                           ln_idx=0)

                # ---- FFN (f1 reuses the o_all tiles) ----
                for mc in range(KC):
                    for hf in range(2):
                        ps = proj_half(w1_sb[l][:], mc, h, hf)
                        nc.vector.tensor_scalar(out=o_all[mc][:, hsl(hf)],
                                                in0=ps[:],
                                                scalar1=bcol("b1", l, mc),
                                                scalar2=0.0, op0=OP.add,
                                                op1=OP.max)
                for mc in range(KC):
                    for hf in range(2):
                        ps = proj_half(w2_sb[l][:], mc, o_all, hf)
                        nc.vector.scalar_tensor_tensor(
                            out=h[mc][:, hsl(hf)], in0=ps[:],
                            scalar=bcol("b2", l, mc),
                            in1=h[mc][:, hsl(hf)], op0=OP.add, op1=OP.add)
                layer_norm(nc, psOZ, psB, smallp, h, ones_col,
                           ones_row128, dcol_row, eps2_row, lnp_sb[l],
                           ln_idx=1)

            # ================= stage 3 =================
            for mc in range(KC):
                ot = bigp.tile([128, SEQ], F32, tag="big", name="ot")
                for hf in range(2):
                    ps = proj_half(win_sb[:], mc, h, hf)
                    nc.scalar.activation(ot[:, hsl(hf)], ps[:], AF.Identity,
                                         bias=bcol("bin", mc=mc))
                nc.sync.dma_start(out_d.ap()[mc * 128:(mc + 1) * 128, :], ot[:])

    nc.compile()
    return nc


def layer_norm(nc, psOZ, psB, smallp, h, ones_col, ones_row128, dcol_row,
               eps2_row, lnp_l, ln_idx):
    """Post-norm LN over the channel (partition) dim of hT [256, SEQ].

    Chunked by 512 tokens; fully on-chip. Per chunk:
      S, SS land on psum partition 0 (cols 0:512 / 512:1024)
      var' = D*SS - S^2 + D^2*eps ;  q = 1/sqrt(var')
      A = D*q  (K=1 matmul with lhsT = D)      [replicated 128 rows]
      B = S*q  (K=1 matmul with lhsT = 1)
      h = ((h*A) - B)*g + b
    """
    for nch in range(NCH):
        nsl = slice(nch * 512, (nch + 1) * 512)
        st = psOZ.tile([128, 1024], F32, tag="psOZ", name="st")
        xsq = smallp.tile([128, 512], F32, tag="lnxsq", name="xsq")
        for k in range(KC):
            nc.scalar.activation(xsq[:], h[k][:, nsl], AF.Square)
            nc.tensor.matmul(st[0:1, 0:512], lhsT=ones_col[:, 0:1],
                             rhs=h[k][:, nsl], start=(k == 0),
                             stop=(k == KC - 1))
            nc.tensor.matmul(st[0:1, 512:1024], lhsT=ones_col[:, 0:1],
                             rhs=xsq[:], start=(k == 0),
                             stop=(k == KC - 1))
        s2 = smallp.tile([1, 512], F32, tag="lns2", name="s2")
        nc.scalar.activation(s2[:], st[0:1, 0:512], AF.Square)
        varp = smallp.tile([1, 512], F32, tag="lnvarp", name="varp")
        nc.vector.scalar_tensor_tensor(out=varp[:], in0=st[0:1, 512:1024],
                                       scalar=float(D), in1=s2[:],
                                       op0=OP.mult, op1=OP.subtract)
        sd = smallp.tile([1, 512], F32, tag="lnsd", name="sd")
        nc.scalar.activation(sd[:], varp[:], AF.Sqrt, bias=eps2_row[0:1, 0:1])
        q = smallp.tile([1, 512], F32, tag="lnq", name="q")
        nc.vector.reciprocal_approx_fast(out=q[:], in_=sd[:])
        brow = smallp.tile([1, 512], F32, tag="lnbrow", name="brow")
        nc.vector.tensor_tensor(out=brow[:], in0=st[0:1, 0:512], in1=q[:],
                                op=OP.mult)
        a_ps = psB.tile([128, 512], F32, tag="psB", name="a_ps")
        nc.tensor.matmul(a_ps[:], lhsT=dcol_row[0:1, :], rhs=q[:],
                         start=True, stop=True)
        b_ps = psB.tile([128, 512], F32, tag="psB", name="b_ps")
        nc.tensor.matmul(b_ps[:], lhsT=ones_row128[0:1, :], rhs=brow[:],
                         start=True, stop=True)
        # apply: h = ((h * A) - B) * g + b
        for k in range(KC):
            t = smallp.tile([128, 512], F32, tag="lnt", name="t")
            nc.vector.tensor_tensor(out=t[:], in0=h[k][:, nsl], in1=a_ps[:],
                                    op=OP.mult)
            nc.vector.tensor_tensor(out=t[:], in0=t[:], in1=b_ps[:],
                                    op=OP.subtract)
            nc.scalar.activation(h[k][:, nsl], t[:], AF.Identity,
                                 bias=lnp_l[:, k, 2 * ln_idx + 1:
                                            2 * ln_idx + 2],
                                 scale=lnp_l[:, k, 2 * ln_idx:2 * ln_idx + 1])


# ================= host side =================

_COMPILED = None


def _get_compiled():
    global _COMPILED
    if _COMPILED is None:
        _COMPILED = build()
    return _COMPILED


def prepare_inputs(inputs):
    """Returns (in_maps, bn2_params) for the 8 cores."""
    f32 = np.float32
    x = np.asarray(inputs["x"], f32)
    adj = np.asarray(inputs["adj_fc"])
    spd = np.asarray(inputs["spd_dist"])
    W_first = np.asarray(inputs["W_first"], f32)
    b_first = np.asarray(inputs["b_first"], f32)
    bn1_g = np.asarray(inputs["bn1_g"], f32)
    bn1_b = np.asarray(inputs["bn1_b"], f32)
    deg_emb = np.asarray(inputs["deg_emb"], f32)
    spd_emb = np.asarray(inputs["spd_emb"], f32)
    Wqkv = np.asarray(inputs["Wqkv"], f32)
    bqkv = np.asarray(inputs["bqkv"], f32)
    Wo = np.asarray(inputs["Wo"], f32)
    bo = np.asarray(inputs["bo"], f32)
    ln1_g = np.asarray(inputs["ln1_g"], f32)
    ln1_b = np.asarray(inputs["ln1_b"], f32)
    W1 = np.asarray(inputs["W1"], f32)
    b1 = np.asarray(inputs["b1"], f32)
    W2 = np.asarray(inputs["W2"], f32)
    b2 = np.asarray(inputs["b2"], f32)
    ln2_g = np.asarray(inputs["ln2_g"], f32)
    ln2_b = np.asarray(inputs["ln2_b"], f32)
    W_in = np.asarray(inputs["W_in"], f32)
    b_in = np.asarray(inputs["b_in"], f32)

    # ---- BN1 stats (exact, host) ----
    Xd = x.astype(np.float64)
    M = Xd.shape[0]
    mu_x = Xd.mean(0)
    C = (Xd.T @ Xd) / M
    Wd = W_first.astype(np.float64)
    m1 = Wd @ mu_x + b_first
    e2 = np.einsum("oc,cd,od->o", Wd, C, Wd)
    v1 = e2 - (Wd @ mu_x) ** 2
    s1 = (bn1_g / np.sqrt(v1 + EPS)).astype(f32)
    Wf_eff = (W_first * s1[:, None]).astype(f32)
    bf_eff = (b_first * s1 + bn1_b - m1.astype(f32) * s1).astype(f32)

    # ---- deg embedding ----
    deg = (adj != 0).sum(1)
    hdeg = deg_emb[deg]                                # [B, N, D]

    # ---- fold v-bias through Wo into bo (softmax rows sum to 1) ----
    bv = bqkv[:, 2 * D:3 * D]                          # [L, D]
    bo_eff = bo + np.einsum("lod,ld->lo", Wo, bv)

    def pack_wT(W):
        WT = np.ascontiguousarray(W.T)                 # [din, dout]
        return WT.reshape(KC, 128, W.shape[0]).transpose(1, 0, 2).copy()

    wfirstT = pack_wT(Wf_eff)
    wqkvT = np.stack([pack_wT(Wqkv[l]) for l in range(L)])
    woT = np.stack([pack_wT(Wo[l]) for l in range(L)])
    w1T = np.stack([pack_wT(W1[l]) for l in range(L)])
    w2T = np.stack([pack_wT(W2[l]) for l in range(L)])
    winT = pack_wT(W_in)

    bcols = np.zeros((128, NBC), f32)
    for mc in range(KC):
        bcols[:, _bc_off("bf", mc=mc)] = bf_eff[mc * 128:(mc + 1) * 128]
        bcols[:, _bc_off("bin", mc=mc)] = b_in[mc * 128:(mc + 1) * 128]
    for l in range(L):
        for mc in range(4):
            bcols[:, _bc_off("qk", l, mc)] = bqkv[l][mc * 128:(mc + 1) * 128]
        for mc in range(KC):
            bcols[:, _bc_off("bo", l, mc)] = bo_eff[l][mc * 128:(mc + 1) * 128]
            bcols[:, _bc_off("b1", l, mc)] = b1[l][mc * 128:(mc + 1) * 128]
            bcols[:, _bc_off("b2", l, mc)] = b2[l][mc * 128:(mc + 1) * 128]

    lnp = np.zeros((L, 128, KC, 4), f32)
    for l in range(L):
        for k in range(KC):
            sl = slice(k * 128, (k + 1) * 128)
            lnp[l, :, k, 0] = ln1_g[l][sl]
            lnp[l, :, k, 1] = ln1_b[l][sl]
            lnp[l, :, k, 2] = ln2_g[l][sl]
            lnp[l, :, k, 3] = ln2_b[l][sl]

    shared = {
        "wfirstT": wfirstT, "wqkvT": wqkvT, "woT": woT, "w1T": w1T,
        "w2T": w2T, "winT": winT, "bcols": bcols, "lnp": lnp,
    }

    # ---- G = exp(bias), [j, i]-transposed, per-core layout ----
    neg = spd < 0
    idx = np.where(neg, 0, spd)
    expT = np.exp(spd_emb)                             # [100, H]
    einv = f32(np.exp(-1.0))

    in_maps = []
    for c in range(NCORES):
        gsl = slice(4 * c * N, 4 * (c + 1) * N)
        xT = np.ascontiguousarray(x[gsl].T)
        hdegT = np.ascontiguousarray(
            hdeg[4 * c:4 * (c + 1)].reshape(SEQ, D).T)
        gb = np.empty((GPC, 2, 4, 128, 2048), ml_dtypes.bfloat16)
        for gl in range(GPC):
            for hh in range(H):
                src = 8 * gl + hh
                val = expT[idx[src], c]                # [N_i, N_j]
                val = np.where(neg[src], einv, val)
                vT = val.T                             # [j, i]
                hg, r = hh // 4, hh % 4
                for jt in range(4):
                    gb[gl, hg, jt, :, r * 512:(r + 1) * 512] = \
                        vT[jt * 128:(jt + 1) * 128, :]
        m = {"xT": xT, "hdegT": hdegT, "gbias": gb}
        m.update(shared)
        in_maps.append(m)

    bn2 = (np.asarray(inputs["bn2_g"], f32), np.asarray(inputs["bn2_b"], f32))
    return in_maps, bn2


def finish_host(results, bn2):
    g2, b2 = bn2
    y2 = np.concatenate([np.ascontiguousarray(r["y2T"].T) for r in results])
    yd = y2.astype(np.float64)
    m = yd.mean(0)
    v = yd.var(0)
    s = (g2 / np.sqrt(v + EPS)).astype(np.float32)
    t = (b2 - m.astype(np.float32) * s)
    out = y2 * s + t
    return np.where(out >= 0, out, np.float32(0.01) * out).astype(np.float32)


def kernel(**inputs):
    nc = _get_compiled()
    in_maps, bn2 = prepare_inputs(inputs)
    res = bass_utils.run_bass_kernel_spmd(
        nc, in_maps, core_ids=list(range(NCORES)))
    return finish_host(res.results, bn2)


# revision 9
# speedup vs baseline: 1.1484x; 1.1149x over previous
"""Graphormer forward on 8 TRN2 NeuronCores (Bass/Tile).

Sharding: data-parallel over graphs, core c -> graphs 4c..4c+3.
Device works in transposed activation layout hT [D=256 (2 chunks of 128), SEQ=2048].

Host precomputes (exact math, no device collectives needed):
  - BN1 stats from X^T X (mean/var of x @ W_first^T are host-computable)
  - deg embedding rows hdeg = deg_emb[(adj!=0).sum(1)]
  - G = exp(attention bias) in [j, i] (transposed) per-core layout, bf16
  - BN2 (final batchnorm + leaky relu) applied on host to the device y2 output
  - v-projection bias folded into bo (softmax weights sum to 1)

Device per core:
  stage1: y = Wfirst' @ xT -> h = lrelu(y + b') + hdegT   [BN1 folded into W']
  2 transformer layers (attention with multiplicative exp-bias G, postnorm LNs)
  stage3: y2T = W_in @ hT + b_in -> DRAM out
"""

import numpy as np
import ml_dtypes

import concourse.bass as bass
import concourse.mybir as mybir
import concourse.tile as tile
from concourse import bacc
from concourse import bass_utils

F32 = mybir.dt.float32
BF16 = mybir.dt.bfloat16
AF = mybir.ActivationFunctionType
OP = mybir.AluOpType

B, N, DIN, D, H, L, DOUT = 32, 512, 256, 256, 8, 2, 256
DH = D // H          # 32
EPS = 1e-5
NCORES = 8
GPC = B // NCORES    # 4 graphs per core
SEQ = GPC * N        # 2048
SCALE = float(1.0 / np.sqrt(DH))
KC = D // 128        # 2 channel chunks
NCH = SEQ // 512     # 4 column chunks of 512

# bias column layout (DRAM "bcols" [128, NBC]):
#   bf: KC | qk: L*4 | bo: L*KC | b1: L*KC | b2: L*KC | bin: KC
def _bc_off(kind, l=0, mc=0):
    if kind == "bf":
        return mc
    if kind == "qk":
        return KC + l * 4 + mc
    if kind == "bo":
        return KC + L * 4 + l * KC + mc
    if kind == "b1":
        return KC + L * 4 + L * KC + l * KC + mc
    if kind == "b2":
        return KC + L * 4 + 2 * L * KC + l * KC + mc
    if kind == "bin":
        return KC + L * 4 + 3 * L * KC + mc
    raise KeyError(kind)


NBC = KC + L * 4 + 3 * L * KC + KC


def build():
    nc = bacc.Bacc("TRN2", target_bir_lowering=False, debug=False,
                   num_devices=NCORES)

    # ---- DRAM I/O ----
    xT_d = nc.dram_tensor("xT", [D, SEQ], F32, kind="ExternalInput")
    hdeg_d = nc.dram_tensor("hdegT", [D, SEQ], F32, kind="ExternalInput")
    g_d = nc.dram_tensor("gbias", [GPC, 2, 4, 128, 2048], BF16,
                         kind="ExternalInput")
    wf_d = nc.dram_tensor("wfirstT", [128, KC, D], F32, kind="ExternalInput")
    wqkv_d = nc.dram_tensor("wqkvT", [L, 128, KC, 3 * D], F32,
                            kind="ExternalInput")
    wo_d = nc.dram_tensor("woT", [L, 128, KC, D], F32, kind="ExternalInput")
    w1_d = nc.dram_tensor("w1T", [L, 128, KC, D], F32, kind="ExternalInput")
    w2_d = nc.dram_tensor("w2T", [L, 128, KC, D], F32, kind="ExternalInput")
    win_d = nc.dram_tensor("winT", [128, KC, D], F32, kind="ExternalInput")
    bc_d = nc.dram_tensor("bcols", [128, NBC], F32, kind="ExternalInput")
    lnp_d = nc.dram_tensor("lnp", [L, 128, KC, 4], F32, kind="ExternalInput")
    out_d = nc.dram_tensor("y2T", [D, SEQ], F32, kind="ExternalOutput")

    with tile.TileContext(nc) as tc:
        with tc.tile_pool(name="const", bufs=1) as constp, \
             tc.tile_pool(name="pers", bufs=1) as pers, \
             tc.tile_pool(name="big", bufs=4) as bigp, \
             tc.tile_pool(name="wpool", bufs=5) as wpool, \
             tc.tile_pool(name="gpool", bufs=5) as gpool, \
             tc.tile_pool(name="zpool", bufs=3) as zpool, \
             tc.tile_pool(name="small", bufs=4) as smallp, \
             tc.tile_pool(name="psS", bufs=2, space="PSUM") as psS, \
             tc.tile_pool(name="psB", bufs=2, space="PSUM") as psB, \
             tc.tile_pool(name="psOZ", bufs=1, space="PSUM") as psOZ:

            # ---- load constants / weights ----
            wf_sb = constp.tile([128, KC, D], F32, tag="wf")
            nc.sync.dma_start(wf_sb[:], wf_d.ap())
            wqkv_sb = [constp.tile([128, KC, 3 * D], F32, tag=f"wqkv{l}",
                                   name=f"wqkv{l}") for l in range(L)]
            wo_sb = [constp.tile([128, KC, D], F32, tag=f"wo{l}",
                                 name=f"wo{l}") for l in range(L)]
            w1_sb = [constp.tile([128, KC, D], F32, tag=f"w1{l}",
                                 name=f"w1{l}") for l in range(L)]
            w2_sb = [constp.tile([128, KC, D], F32, tag=f"w2{l}",
                                 name=f"w2{l}") for l in range(L)]
            for l in range(L):
                nc.sync.dma_start(wqkv_sb[l][:], wqkv_d.ap()[l])
                nc.sync.dma_start(wo_sb[l][:], wo_d.ap()[l])
                nc.sync.dma_start(w1_sb[l][:], w1_d.ap()[l])
                nc.sync.dma_start(w2_sb[l][:], w2_d.ap()[l])
            win_sb = constp.tile([128, KC, D], F32, tag="win")
            nc.sync.dma_start(win_sb[:], win_d.ap())
            bc_sb = constp.tile([128, NBC], F32, tag="bc")
            nc.sync.dma_start(bc_sb[:], bc_d.ap())
            lnp_sb = [constp.tile([128, KC, 4], F32, tag=f"lnp{l}",
                                  name=f"lnp{l}") for l in range(L)]
            for l in range(L):
                nc.sync.dma_start(lnp_sb[l][:], lnp_d.ap()[l])

            ones_col = constp.tile([128, 1], F32, tag="ones_col")
            nc.vector.memset(ones_col[:], 1.0)
            ones_mat_bf = constp.tile([128, 32], BF16, tag="ones_mat_bf")
            nc.vector.memset(ones_mat_bf[:], 1.0)
            ones_row128 = constp.tile([1, 128], F32, tag="ones_row128")
            nc.vector.memset(ones_row128[:], 1.0)
            dcol_row = constp.tile([1, 128], F32, tag="dcol_row")
            nc.vector.memset(dcol_row[:], float(D))
            eps2_row = constp.tile([1, 1], F32, tag="eps2_row")
            nc.vector.memset(eps2_row[:], float(D) * float(D) * EPS)

            def bcol(kind, l=0, mc=0):
                o = _bc_off(kind, l, mc)
                return bc_sb[:, o:o + 1]

            # ---- persistent activations ----
            h = [pers.tile([128, SEQ], F32, tag=f"h{k}", name=f"h{k}")
                 for k in range(KC)]
            qkT = [pers.tile([128, SEQ], BF16, tag=f"qk{m}", name=f"qk{m}")
                   for m in range(4)]
            v_norm = [pers.tile([128, D], BF16, tag=f"v{s}", name=f"v{s}")
                      for s in range(16)]
            # o_all doubles as the FFN hidden buffer (disjoint lifetimes)
            o_all = [pers.tile([128, SEQ], F32, tag=f"oall{k}",
                               name=f"oall{k}") for k in range(KC)]

            def proj_half(wT, mc, rhs_tiles, half):
                """psS tile [128, 1024] = wT[:, :, mc*128:..].T @ rhs[half]."""
                ps = psS.tile([128, 1024], F32, tag="psS", name="ps")
                msl = slice(mc * 128, (mc + 1) * 128)
                for nch in range(2):
                    base = half * 1024 + nch * 512
                    nsl = slice(base, base + 512)
                    psl = slice(nch * 512, (nch + 1) * 512)
                    for k in range(KC):
                        nc.tensor.matmul(ps[:, psl], lhsT=wT[:, k, msl],
                                         rhs=rhs_tiles[k][:, nsl],
                                         start=(k == 0), stop=(k == KC - 1))
                return ps

            def hsl(half):
                return slice(half * 1024, (half + 1) * 1024)

            # ================= stage 1 =================
            xin = [bigp.tile([128, SEQ], F32, tag="big", name="xin")
                   for _ in range(KC)]
            hdeg_sb = [bigp.tile([128, SEQ], F32, tag="big", name="hdeg")
                       for _ in range(KC)]
            for k in range(KC):
                nc.sync.dma_start(xin[k][:], xT_d.ap()[k * 128:(k + 1) * 128, :])
                nc.sync.dma_start(hdeg_sb[k][:],
                                  hdeg_d.ap()[k * 128:(k + 1) * 128, :])
            for mc in range(KC):
                for hf in range(2):
                    ps = proj_half(wf_sb[:], mc, xin, hf)
                    nc.scalar.activation(h[mc][:, hsl(hf)], ps[:], AF.Lrelu,
                                         bias=bcol("bf", mc=mc), alpha=0.01)
                    nc.vector.tensor_tensor(out=h[mc][:, hsl(hf)],
                                            in0=h[mc][:, hsl(hf)],
                                            in1=hdeg_sb[mc][:, hsl(hf)],
                                            op=OP.add)

            # ================= transformer layers =================
            for l in range(L):
                # ---- q^T, k^T (transposed layout, bf16) ----
                for mc in range(4):
                    for hf in range(2):
                        ps = proj_half(wqkv_sb[l][:], mc, h, hf)
                        nc.any.tensor_scalar(out=qkT[mc][:, hsl(hf)],
                                             in0=ps[:],
                                             scalar1=bcol("qk", l, mc),
                                             scalar2=None, op0=OP.add)
                # ---- v (seq-major, bf16; bias folded into bo on host) ----
                for st in range(16):
                    ssl = slice(st * 128, (st + 1) * 128)
                    ps = psB.tile([128, D], F32, tag="psB", name="psv")
                    for k in range(KC):
                        nc.tensor.matmul(ps[:], lhsT=h[k][:, ssl],
                                         rhs=wqkv_sb[l][:, k, 2 * D:3 * D],
                                         start=(k == 0), stop=(k == KC - 1))
                    nc.any.tensor_copy(out=v_norm[st][:], in_=ps[:])

                # ---- attention ----
                for g in range(GPC):
                    gs = slice(g * 512, (g + 1) * 512)
                    for hg in range(2):
                        oz = psOZ.tile([128, 1024], F32, tag="psOZ", name="oz")
                        for jt in range(4):
                            jsl = slice(g * 512 + jt * 128,
                                        g * 512 + jt * 128 + 128)
                            for rr in range(2):   # 2-head substeps
                                sc = psS.tile([128, 1024], F32, tag="psS",
                                              name="sc")
                                w_sb = wpool.tile([128, 1024], BF16, tag="w",
                                                  name="w_sb")
                                for q in range(2):
                                    r = rr * 2 + q
                                    qt = hg        # q^T tile index
                                    kt = 2 + hg    # k^T tile index
                                    rsl = slice(r * 32, (r + 1) * 32)
                                    nc.tensor.matmul(
                                        sc[:, q * 512:(q + 1) * 512],
                                        lhsT=qkT[kt][rsl, jsl],
                                        rhs=qkT[qt][rsl, gs],
                                        start=True, stop=True,
                                        tile_position=(r * 32, 0))
                                nc.scalar.activation(w_sb[:], sc[:], AF.Exp,
                                                     scale=SCALE)
                                g_sb = gpool.tile([128, 1024], BF16, tag="g",
                                                  name="g_sb")
                                nc.sync.dma_start(
                                    g_sb[:],
                                    g_d.ap()[g, hg, jt][:,
                                        rr * 1024:(rr + 1) * 1024])
                                nc.vector.tensor_tensor(out=w_sb[:],
                                                        in0=w_sb[:],
                                                        in1=g_sb[:],
                                                        op=OP.mult)
                                for q in range(2):
                                    r = rr * 2 + q
                                    rssl = slice(q * 512, (q + 1) * 512)
                                    vsl = slice((hg * 4 + r) * 32,
                                                (hg * 4 + r) * 32 + 32)
                                    nc.tensor.matmul(
                                        oz[r * 32:(r + 1) * 32, 0:512],
                                        lhsT=v_norm[g * 4 + jt][:, vsl],
                                        rhs=w_sb[:, rssl],
                                        start=(jt == 0), stop=(jt == 3),
                                        tile_position=(0, r * 32))
                                    nc.tensor.matmul(
                                        oz[r * 32:(r + 1) * 32, 512:1024],
                                        lhsT=ones_mat_bf[:, 0:32],
                                        rhs=w_sb[:, rssl],
                                        start=(jt == 0), stop=(jt == 3),
                                        tile_position=(0, r * 32))
                        # normalize 4 heads: the Z matmul used a ones MATRIX
                        # lhsT, so oz[:, 512:1024] already holds Z replicated
                        # across each head's 32 rows.
                        zc = zpool.tile([128, 512], F32, tag="zc", name="zc")
                        nc.any.tensor_copy(out=zc[:], in_=oz[:, 512:1024])
                        rz = zpool.tile([128, 512], F32, tag="rz", name="rz")
                        nc.vector.reciprocal_approx_fast(out=rz[:], in_=zc[:])
                        nc.vector.tensor_tensor(out=o_all[hg][:, gs],
                                                in0=oz[:, 0:512], in1=rz[:],
                                                op=OP.mult)

                # ---- attn out proj + residual + LN1 ----
                for mc in range(KC):
                    for hf in range(2):
                        ps = proj_half(wo_sb[l][:], mc, o_all, hf)
                        nc.vector.scalar_tensor_tensor(
                            out=h[mc][:, hsl(hf)], in0=ps[:],
                            scalar=bcol("bo", l, mc),
                            in1=h[mc][:, hsl(hf)], op0=OP.add, op1=OP.add)
                layer_norm(nc, psS, psB, smallp, h, ones_col,
                           ones_row128, dcol_row, eps2_row, lnp_sb[l],
                           ln_idx=0)

                # ---- FFN (f1 reuses the o_all tiles) ----
                for mc in range(KC):
                    for hf in range(2):
                        ps = proj_half(w1_sb[l][:], mc, h, hf)
                        nc.vector.tensor_scalar(out=o_all[mc][:, hsl(hf)],
                                                in0=ps[:],
                                                scalar1=bcol("b1", l, mc),
                                                scalar2=0.0, op0=OP.add,
                                                op1=OP.max)
                for mc in range(KC):
                    for hf in range(2):
                        ps = proj_half(w2_sb[l][:], mc, o_all, hf)
                        nc.vector.scalar_tensor_tensor(
                            out=h[mc][:, hsl(hf)], in0=ps[:],
                            scalar=bcol("b2", l, mc),
                            in1=h[mc][:, hsl(hf)], op0=OP.add, op1=OP.add)
                layer_norm(nc, psS, psB, smallp, h, ones_col,
                           ones_row128, dcol_row, eps2_row, lnp_sb[l],
                           ln_idx=1)

            # ================= stage 3 =================
            for mc in range(KC):
                ot = bigp.tile([128, SEQ], F32, tag="big", name="ot")
                for hf in range(2):
                    ps = proj_half(win_sb[:], mc, h, hf)
                    nc.scalar.activation(ot[:, hsl(hf)], ps[:], AF.Identity,
                                         bias=bcol("bin", mc=mc))
                nc.sync.dma_start(out_d.ap()[mc * 128:(mc + 1) * 128, :], ot[:])

    nc.compile()
    return nc


def layer_norm(nc, psS, psB, smallp, h, ones_col, ones_row128, dcol_row,
               eps2_row, lnp_l, ln_idx):
    """Post-norm LN over the channel (partition) dim of hT [256, SEQ].

    Chunked by 512 tokens; fully on-chip. Per chunk:
      S, SS land on psum partition 0 (cols 0:512 / 512:1024)
      var' = D*SS - S^2 + D^2*eps ;  q = 1/sqrt(var')
      A = D*q  (K=1 matmul with lhsT = D)      [replicated 128 rows]
      B = S*q  (K=1 matmul with lhsT = 1)
      h = ((h*A) - B)*g + b
    """
    for nch in range(NCH):
        nsl = slice(nch * 512, (nch + 1) * 512)
        st = psS.tile([128, 1024], F32, tag="psS", name="st")
        xsq = smallp.tile([128, 512], F32, tag="lnxsq", name="xsq")
        for k in range(KC):
            nc.scalar.activation(xsq[:], h[k][:, nsl], AF.Square)
            nc.tensor.matmul(st[0:1, 0:512], lhsT=ones_col[:, 0:1],
                             rhs=h[k][:, nsl], start=(k == 0),
                             stop=(k == KC - 1))
            nc.tensor.matmul(st[0:1, 512:1024], lhsT=ones_col[:, 0:1],
                             rhs=xsq[:], start=(k == 0),
                             stop=(k == KC - 1))
        s2 = smallp.tile([1, 512], F32, tag="lns2", name="s2")
        nc.scalar.activation(s2[:], st[0:1, 0:512], AF.Square)
        varp = smallp.tile([1, 512], F32, tag="lnvarp", name="varp")
        nc.vector.scalar_tensor_tensor(out=varp[:], in0=st[0:1, 512:1024],
                                       scalar=float(D), in1=s2[:],
                                       op0=OP.mult, op1=OP.subtract)
        sd = smallp.tile([1, 512], F32, tag="lnsd", name="sd")
        nc.scalar.activation(sd[:], varp[:], AF.Sqrt, bias=eps2_row[0:1, 0:1])
        q = smallp.tile([1, 512], F32, tag="lnq", name="q")
        nc.vector.reciprocal_approx_fast(out=q[:], in_=sd[:])
        brow = smallp.tile([1, 512], F32, tag="lnbrow", name="brow")
        nc.vector.tensor_tensor(out=brow[:], in0=st[0:1, 0:512], in1=q[:],
                                op=OP.mult)
        a_ps = psB.tile([128, 512], F32, tag="psB", name="a_ps")
        nc.tensor.matmul(a_ps[:], lhsT=dcol_row[0:1, :], rhs=q[:],
                         start=True, stop=True)
        b_ps = psB.tile([128, 512], F32, tag="psB", name="b_ps")
        nc.tensor.matmul(b_ps[:], lhsT=ones_row128[0:1, :], rhs=brow[:],
                         start=True, stop=True)
        # apply: h = ((h * A) - B) * g + b
        for k in range(KC):
            t = smallp.tile([128, 512], F32, tag="lnt", name="t")
            nc.vector.tensor_tensor(out=t[:], in0=h[k][:, nsl], in1=a_ps[:],
                                    op=OP.mult)
            nc.vector.tensor_tensor(out=t[:], in0=t[:], in1=b_ps[:],
                                    op=OP.subtract)
            nc.scalar.activation(h[k][:, nsl], t[:], AF.Identity,
                                 bias=lnp_l[:, k, 2 * ln_idx + 1:
                                            2 * ln_idx + 2],
                                 scale=lnp_l[:, k, 2 * ln_idx:2 * ln_idx + 1])


# ================= host side =================

_COMPILED = None


def _get_compiled():
    global _COMPILED
    if _COMPILED is None:
        _COMPILED = build()
    return _COMPILED


def prepare_inputs(inputs):
    """Returns (in_maps, bn2_params) for the 8 cores."""
    f32 = np.float32
    x = np.asarray(inputs["x"], f32)
    adj = np.asarray(inputs["adj_fc"])
    spd = np.asarray(inputs["spd_dist"])
    W_first = np.asarray(inputs["W_first"], f32)
    b_first = np.asarray(inputs["b_first"], f32)
    bn1_g = np.asarray(inputs["bn1_g"], f32)
    bn1_b = np.asarray(inputs["bn1_b"], f32)
    deg_emb = np.asarray(inputs["deg_emb"], f32)
    spd_emb = np.asarray(inputs["spd_emb"], f32)
    Wqkv = np.asarray(inputs["Wqkv"], f32)
    bqkv = np.asarray(inputs["bqkv"], f32)
    Wo = np.asarray(inputs["Wo"], f32)
    bo = np.asarray(inputs["bo"], f32)
    ln1_g = np.asarray(inputs["ln1_g"], f32)
    ln1_b = np.asarray(inputs["ln1_b"], f32)
    W1 = np.asarray(inputs["W1"], f32)
    b1 = np.asarray(inputs["b1"], f32)
    W2 = np.asarray(inputs["W2"], f32)
    b2 = np.asarray(inputs["b2"], f32)
    ln2_g = np.asarray(inputs["ln2_g"], f32)
    ln2_b = np.asarray(inputs["ln2_b"], f32)
    W_in = np.asarray(inputs["W_in"], f32)
    b_in = np.asarray(inputs["b_in"], f32)

    # ---- BN1 stats (exact, host) ----
    Xd = x.astype(np.float64)
    M = Xd.shape[0]
    mu_x = Xd.mean(0)
    C = (Xd.T @ Xd) / M
    Wd = W_first.astype(np.float64)
    m1 = Wd @ mu_x + b_first
    e2 = np.einsum("oc,cd,od->o", Wd, C, Wd)
    v1 = e2 - (Wd @ mu_x) ** 2
    s1 = (bn1_g / np.sqrt(v1 + EPS)).astype(f32)
    Wf_eff = (W_first * s1[:, None]).astype(f32)
    bf_eff = (b_first * s1 + bn1_b - m1.astype(f32) * s1).astype(f32)

    # ---- deg embedding ----
    deg = (adj != 0).sum(1)
    hdeg = deg_emb[deg]                                # [B, N, D]

    # ---- fold v-bias through Wo into bo (softmax rows sum to 1) ----
    bv = bqkv[:, 2 * D:3 * D]                          # [L, D]
    bo_eff = bo + np.einsum("lod,ld->lo", Wo, bv)

    def pack_wT(W):
        WT = np.ascontiguousarray(W.T)                 # [din, dout]
        return WT.reshape(KC, 128, W.shape[0]).transpose(1, 0, 2).copy()

    wfirstT = pack_wT(Wf_eff)
    wqkvT = np.stack([pack_wT(Wqkv[l]) for l in range(L)])
    woT = np.stack([pack_wT(Wo[l]) for l in range(L)])
    w1T = np.stack([pack_wT(W1[l]) for l in range(L)])
    w2T = np.stack([pack_wT(W2[l]) for l in range(L)])
    winT = pack_wT(W_in)

    bcols = np.zeros((128, NBC), f32)
    for mc in range(KC):
        bcols[:, _bc_off("bf", mc=mc)] = bf_eff[mc * 128:(mc + 1) * 128]
        bcols[:, _bc_off("bin", mc=mc)] = b_in[mc * 128:(mc + 1) * 128]
    for l in range(L):
        for mc in range(4):
            bcols[:, _bc_off("qk", l, mc)] = bqkv[l][mc * 128:(mc + 1) * 128]
        for mc in range(KC):
            bcols[:, _bc_off("bo", l, mc)] = bo_eff[l][mc * 128:(mc + 1) * 128]
            bcols[:, _bc_off("b1", l, mc)] = b1[l][mc * 128:(mc + 1) * 128]
            bcols[:, _bc_off("b2", l, mc)] = b2[l][mc * 128:(mc + 1) * 128]

    lnp = np.zeros((L, 128, KC, 4), f32)
    for l in range(L):
        for k in range(KC):
            sl = slice(k * 128, (k + 1) * 128)
            lnp[l, :, k, 0] = ln1_g[l][sl]
            lnp[l, :, k, 1] = ln1_b[l][sl]
            lnp[l, :, k, 2] = ln2_g[l][sl]
            lnp[l, :, k, 3] = ln2_b[l][sl]

    shared = {
        "wfirstT": wfirstT, "wqkvT": wqkvT, "woT": woT, "w1T": w1T,
        "w2T": w2T, "winT": winT, "bcols": bcols, "lnp": lnp,
    }

    # ---- G = exp(bias), [j, i]-transposed, per-core layout ----
    neg = spd < 0
    idx = np.where(neg, 0, spd)
    expT = np.exp(spd_emb)                             # [100, H]
    einv = f32(np.exp(-1.0))

    in_maps = []
    for c in range(NCORES):
        gsl = slice(4 * c * N, 4 * (c + 1) * N)
        xT = np.ascontiguousarray(x[gsl].T)
        hdegT = np.ascontiguousarray(
            hdeg[4 * c:4 * (c + 1)].reshape(SEQ, D).T)
        gb = np.empty((GPC, 2, 4, 128, 2048), ml_dtypes.bfloat16)
        for gl in range(GPC):
            for hh in range(H):
                src = 8 * gl + hh
                val = expT[idx[src], c]                # [N_i, N_j]
                val = np.where(neg[src], einv, val)
                vT = val.T                             # [j, i]
                hg, r = hh // 4, hh % 4
                for jt in range(4):
                    gb[gl, hg, jt, :, r * 512:(r + 1) * 512] = \
                        vT[jt * 128:(jt + 1) * 128, :]
        m = {"xT": xT, "hdegT": hdegT, "gbias": gb}
        m.update(shared)
        in_maps.append(m)

    bn2 = (np.asarray(inputs["bn2_g"], f32), np.asarray(inputs["bn2_b"], f32))
    return in_maps, bn2


def finish_host(results, bn2):
    g2, b2 = bn2
    y2 = np.concatenate([np.ascontiguousarray(r["y2T"].T) for r in results])
    yd = y2.astype(np.float64)
    m = yd.mean(0)
    v = yd.var(0)
    s = (g2 / np.sqrt(v + EPS)).astype(np.float32)
    t = (b2 - m.astype(np.float32) * s)
    out = y2 * s + t
    return np.where(out >= 0, out, np.float32(0.01) * out).astype(np.float32)


def kernel(**inputs):
    nc = _get_compiled()
    in_maps, bn2 = prepare_inputs(inputs)
    res = bass_utils.run_bass_kernel_spmd(
        nc, in_maps, core_ids=list(range(NCORES)))
    return finish_host(res.results, bn2)
